# revision 12
# baseline (speedup 1.0000x reference)
"""Trainium2 Bass kernel for PixelPropagationModule (per-pixel self-attention).

Math per batch sample b (B=8, C=256, CI=64, N=H*W=3136):
    Q = Wq @ x + bq            [CI, N]
    K = Wk @ x + bk            [CI, N]
    V = Wv @ x                 [C,  N]   (bias folded out, see below)
    score[i, j] = sum_o Q[o, i] K[o, j]          (N x N)
    att = softmax(score, axis=j)
    out = gamma * (V @ att^T) + (x + gamma*bv)   -> [C, N]

Since softmax rows sum to 1, the V bias contributes exactly gamma*bv to
every output pixel, so it is folded into the residual tensor on the host
(xf := x + gamma*bv) and V is computed bias-free on chip.

Sharding: pure data parallel, one sample per NeuronCore (B == 8 == n_cores).

Device dataflow (per core):
  - Everything is computed in the "transposed score" orientation S^T[j, i] so
    the attention weights come out of the PE array with j (the contraction
    index of the second matmul) on partitions; no on-chip transposes.
  - S^T matmuls contract over CI=64, which leaves half the 128-row PE array
    idle.  Fix: keep duplicated copies of Q and K on partitions 64..127
    (cheap SBUF->SBUF DMA) and row-tile two j-tiles concurrently - matmul A
    on PE rows 0..63 (tile_position (0,0)) computes j-tile 2s, matmul B on
    rows 64..127 (tile_position (64,0)) computes j-tile 2s+1, into different
    PSUM banks.  2x effective S throughput.
  - Q and K projections are col-tiled (Q -> psum partitions 0:64, K -> 64:128
    concurrently) since each has only M=64 output channels.
  - softmax without max subtraction (|score| <= ~40, exp safe in fp32/bf16);
    denominator s_i accumulated with DVE adds of the exp'ed tiles plus a
    final ones-vector matmul partition-reduce; normalization 1/s_i applied
    to the [C, N] output (deferred, flash-attention style).
  - exp on ACT reads PSUM pairs [128, 2, 448] (two j-tiles) per call to
    amortize the per-instruction overhead.

PSUM layout: S pair tiles are [128, 2, 448] with slots at element offsets
0 and 512 (bank-aligned, a matmul output must not cross a 2KiB psum bank);
out accumulators are [128, 1024] with the two C-halves at offsets 0 / 512.
"""

import numpy as np
import ml_dtypes

import bass_rust as _bass_rust

import concourse.bass as bass
import concourse.mybir as mybir
import concourse.tile as tile
from concourse.bass_utils import run_bass_kernel_spmd

BF16 = mybir.dt.bfloat16
F32 = mybir.dt.float32
NP_BF16 = ml_dtypes.bfloat16
AF = mybir.ActivationFunctionType

B, C, H, W = 8, 256, 56, 56
CI = 64
N = H * W            # 3136
NCORES = 8
FD = 448             # i-chunk width: 7 * 448 = 3136
OFF2 = 512           # second-slot offset inside [128, 1024] psum tiles
NJ = 25              # j-tiles: 24 x 128 + 1 x 64
NPAIR = 12           # full pairs of 128-wide j-tiles


def build_kernel(n_repeat: int = 1) -> bass.Bass:
    nc = bass.Bass()

    xb_d = nc.declare_dram_parameter("xb", [C, N], BF16, isOutput=False)
    xf_d = nc.declare_dram_parameter("xf", [C, N], F32, isOutput=False)
    wq_d = nc.declare_dram_parameter("wqT", [C, CI], BF16, isOutput=False)
    wk_d = nc.declare_dram_parameter("wkT", [C, CI], BF16, isOutput=False)
    wv_d = nc.declare_dram_parameter("wvT", [C, C], BF16, isOutput=False)
    bq_d = nc.declare_dram_parameter("bq", [CI, 1], F32, isOutput=False)
    bk_d = nc.declare_dram_parameter("bk", [CI, 1], F32, isOutput=False)
    out_d = nc.declare_dram_parameter("out", [C, N], F32, isOutput=True)

    xb_r = xb_d[:].rearrange("(o p) n -> p o n", p=128)    # [128, 2, N] bf16
    xf_r = xf_d[:].rearrange("(o p) n -> p o n", p=128)    # [128, 2, N] f32
    out_r = out_d[:].rearrange("(o p) n -> p o n", p=128)  # [128, 2, N] f32

    with tile.TileContext(nc) as tc:
        with (
            tc.tile_pool(name="const", bufs=1) as cpool,
            tc.tile_pool(name="data", bufs=1) as dpool,
            tc.tile_pool(name="att", bufs=6) as apool,
            tc.tile_pool(name="accp", bufs=2) as accpool,
            tc.tile_pool(name="outp", bufs=3) as opool,
            tc.tile_pool(name="misc", bufs=3) as mpool,
            tc.tile_pool(name="ps_a", bufs=2, space="PSUM") as ps_a,
            tc.tile_pool(name="ps_o", bufs=2, space="PSUM") as ps_o,
        ):
            # ---- constants / weights ----
            wq_sb = cpool.tile([128, 2, CI], BF16, name="wq_sb")
            nc.sync.dma_start(wq_sb[:], wq_d[:].rearrange("(o p) m -> p o m", p=128))
            wk_sb = cpool.tile([128, 2, CI], BF16, name="wk_sb")
            nc.sync.dma_start(wk_sb[:], wk_d[:].rearrange("(o p) m -> p o m", p=128))
            wv_sb = cpool.tile([128, 2, C], BF16, name="wv_sb")
            nc.sync.dma_start(wv_sb[:], wv_d[:].rearrange("(o p) m -> p o m", p=128))
            # Q bias on partitions 0:64, K bias on 64:128 — matching where
            # the col-tiled projection psum lives (ACT cannot shift lanes)
            bqk_sb = cpool.tile([128, 1], F32, name="bqk_sb")
            nc.sync.dma_start(bqk_sb[0:CI, :], bq_d[:])
            nc.sync.dma_start(bqk_sb[CI:128, :], bk_d[:])
            ones_col = cpool.tile([128, 1], BF16, name="ones_col")
            nc.vector.memset(ones_col[:], 1.0)
            ones_rb = cpool.tile([1, 128], BF16, name="ones_rb")
            nc.vector.memset(ones_rb[:], 1.0)

            # ---- x in SBUF (chunked so projections start early) ----
            xb_sb = dpool.tile([128, 2, N], BF16, name="xb_sb")
            xb_edges = [0, 112, 224, 448] + [448 * t for t in range(2, 8)]
            for e0, e1 in zip(xb_edges[:-1], xb_edges[1:]):
                nc.sync.dma_start(xb_sb[:, :, e0:e1], xb_r[:, :, e0:e1])
            xf_sb = dpool.tile([128, 2, N], F32, name="xf_sb")

            # warm the PE HAM clock gate during the initial x DMA wait:
            # dummy matmuls on a zeroed scratch tile (results never read)
            warm_sb = cpool.tile([128, 512], BF16, name="warm_sb")
            nc.vector.memset(warm_sb[:], 0.0)
            pwarm = ps_a.tile([128, 1024], F32, tag="ps_a")
            for wi in range(14):
                nc.tensor.matmul(pwarm[:, 0:512], lhsT=warm_sb[:, 0:128],
                                 rhs=warm_sb[:], start=True, stop=True)

            # residual input: needed only from the first chunk's tail on,
            # so emit after xb so it does not steal early DMA bandwidth
            nc.sync.dma_start(xf_sb[:], xf_r)

            # q2/k2: [128, N] with the projection on partitions 0:64 and a
            # DMA-duplicated copy on partitions 64:128 (for S row-tiling)
            q2_sb = dpool.tile([128, N], BF16, name="q2_sb")
            k2_sb = dpool.tile([128, N], BF16, name="k2_sb")
            vt_sb = dpool.tile([128, NJ, C], BF16, name="vt_sb")

            for _rep in range(n_repeat):
                # ---- projections, interleaved by x-DMA arrival ----
                # Q and K are col-tiled into one PE slot per x-half (Q ->
                # psum partitions 0:64, K -> 64:128); V^T tiles fully covered
                # by x columns [0, (t+1)*448) are emitted together so the PE
                # always has ready work while later x chunks stream in.
                vt_done = 0
                for t in range(N // FD):
                    sl = slice(t * FD, (t + 1) * FD)
                    pq = ps_a.tile([128, 1024], F32, tag="ps_a")
                    # Q at psum partitions 0:64 (cols 0:64), K at 64:128
                    nc.tensor.matmul(pq[0:CI, 0:FD], lhsT=wq_sb[:, 0, :],
                                     rhs=xb_sb[:, 0, sl], start=True, stop=False)
                    nc.tensor.matmul(pq[CI:128, 0:FD], lhsT=wk_sb[:, 0, :],
                                     rhs=xb_sb[:, 0, sl], start=True, stop=False)
                    nc.tensor.matmul(pq[0:CI, 0:FD], lhsT=wq_sb[:, 1, :],
                                     rhs=xb_sb[:, 1, sl], start=False, stop=True)
                    nc.tensor.matmul(pq[CI:128, 0:FD], lhsT=wk_sb[:, 1, :],
                                     rhs=xb_sb[:, 1, sl], start=False, stop=True)
                    # bias-apply on DVE (per-partition scalar AP), not ACT —
                    # ACT is the exp bottleneck, DVE has slack here
                    nc.vector.tensor_scalar_add(q2_sb[0:CI, sl],
                                                pq[0:CI, 0:FD], bqk_sb[0:CI, :])
                    nc.vector.tensor_scalar_add(k2_sb[CI:128, sl],
                                                pq[CI:128, 0:FD],
                                                bqk_sb[CI:128, :])
                    # duplicate across partition halves for S row-tiling
                    nc.sync.dma_start(q2_sb[CI:128, sl], q2_sb[0:CI, sl])
                    nc.sync.dma_start(k2_sb[0:CI, sl], k2_sb[CI:128, sl])
                    # V^T tiles: vt_sb[p, jt, c] = gamma*V[c, jt*128+p]
                    vt_avail = min(NJ, ((t + 1) * FD) // 128) if t < N // FD - 1 else NJ
                    for jt in range(vt_done, vt_avail):
                        jsz = 128 if jt < NJ - 1 else 64
                        j0 = jt * 128
                        pv = ps_a.tile([128, 1024], F32, tag="ps_a")
                        pvt = pv[:jsz, 0:C]
                        nc.tensor.matmul(pvt, lhsT=xb_sb[:, 0, j0:j0 + jsz],
                                         rhs=wv_sb[:, 0, :], start=True, stop=False)
                        nc.tensor.matmul(pvt, lhsT=xb_sb[:, 1, j0:j0 + jsz],
                                         rhs=wv_sb[:, 1, :], start=False, stop=True)
                        nc.vector.tensor_copy(vt_sb[:jsz, jt, :], pvt)
                    vt_done = vt_avail

                # ---- attention, one 448-wide query chunk at a time ----
                # S^T j-tiles are produced in row-tiled pairs: matmul A on PE
                # rows 0:64 -> psum slot 0, matmul B on rows 64:128 (the
                # duplicated q/k copies) -> psum slot 1 (different bank).
                # exp processes the pair in one ACT call.
                for ci in range(N // FD):
                    isl = slice(ci * FD, (ci + 1) * FD)
                    po = ps_o.tile([128, 1024], F32, tag="ps_o", name="po")
                    acc = accpool.tile([128, FD], BF16, tag="acc")

                    def emit_out_mms(jt2, att):
                        # att: [128, 2, FD] pair (j-tiles jt2, jt2+1) or
                        # [64, FD] single (last tile)
                        last = jt2 >= NJ - 1
                        if jt2 < NPAIR * 2:
                            for h in range(2):
                                jt = jt2 + h
                                for cc in range(2):
                                    vst = vt_sb[:, jt, cc * 128:(cc + 1) * 128]
                                    nc.tensor.matmul(
                                        po[:, cc * OFF2:cc * OFF2 + FD],
                                        lhsT=vst, rhs=att[:, h, :],
                                        start=(jt == 0), stop=(last and h == 1))
                        else:
                            for cc in range(2):
                                vst = vt_sb[:64, NJ - 1, cc * 128:(cc + 1) * 128]
                                nc.tensor.matmul(
                                    po[:, cc * OFF2:cc * OFF2 + FD],
                                    lhsT=vst, rhs=att[:64, 0, :],
                                    start=False, stop=True)

                    # software pipeline depth 2: out-matmuls of pair s are
                    # emitted after the S-matmuls of pair s+2, so the PE FIFO
                    # never waits on exp.
                    pending = []
                    for s in range(NPAIR + 1):
                        ps = ps_a.tile([128, 1024], F32, tag="ps_a")
                        if s < NPAIR:
                            jA, jB = 2 * s * 128, (2 * s + 1) * 128
                            # row-tiled concurrent pair (disjoint PE rows)
                            nc.tensor.matmul(ps[:, 0:FD],
                                             lhsT=k2_sb[0:64, jA:jA + 128],
                                             rhs=q2_sb[0:64, isl],
                                             start=True, stop=True)
                            nc.tensor.matmul(ps[:, OFF2:OFF2 + FD],
                                             lhsT=k2_sb[64:128, jB:jB + 128],
                                             rhs=q2_sb[64:128, isl],
                                             start=True, stop=True)
                            att = apool.tile([128, 2, FD], BF16, tag="att")
                        else:
                            # last j-tile (64 wide), single matmul
                            nc.tensor.matmul(ps[:64, 0:FD],
                                             lhsT=k2_sb[0:64, NPAIR * 256:N],
                                             rhs=q2_sb[0:64, isl],
                                             start=True, stop=True)
                            att = apool.tile([128, 2, FD], BF16, tag="att")
                        if len(pending) >= 2:
                            emit_out_mms(*pending.pop(0))
                        if s < NPAIR:
                            psv = ps[:].rearrange("p (h x) -> p h x", h=2)[:, :, 0:FD]
                            nc.scalar.activation(att[:], psv, AF.Exp)
                            # softmax denominator accumulation (DVE, 2x bf16)
                            if s == 0:
                                nc.vector.tensor_add(acc[:], att[:, 0, :],
                                                     att[:, 1, :])
                            else:
                                nc.vector.tensor_add(acc[:], acc[:], att[:, 0, :])
                                nc.vector.tensor_add(acc[:], acc[:], att[:, 1, :])
                            pending.append((2 * s, att))
                        else:
                            nc.scalar.activation(att[:64, 0, :], ps[:64, 0:FD],
                                                 AF.Exp)
                            att_last = att
                            pending.append((NJ - 1, att))
                    for p in pending:
                        emit_out_mms(*p)

                    # ---- softmax denominator + normalization + residual ----
                    out_sb = opool.tile([128, 2, OFF2], F32, tag="out")
                    # plain copies first: releases the po psum banks fast so
                    # the next chunk's out-matmuls can start
                    for cc in range(2):
                        nc.vector.tensor_copy(out_sb[:, cc, :FD],
                                              po[:, cc * OFF2:cc * OFF2 + FD])
                    ps1 = ps_o.tile([128, 1024], F32, tag="ps_o", name="ps1")
                    s1 = ps1[:1, 0:FD]
                    nc.tensor.matmul(s1, lhsT=ones_col[:], rhs=acc[:],
                                     start=True, stop=False)
                    nc.tensor.matmul(s1, lhsT=ones_col[:64],
                                     rhs=att_last[:64, 0, :],
                                     start=False, stop=True)
                    inv_sb = mpool.tile([1, OFF2], F32, tag="inv")
                    nc.vector.reciprocal(inv_sb[:, :FD], s1)
                    # bf16 copy of 1/s so the broadcast matmul runs at
                    # 1 cyc/row (fp32 operands cost 4 cyc/row on PE)
                    invb_sb = mpool.tile([1, OFF2], BF16, tag="invb")
                    nc.vector.tensor_copy(invb_sb[:, :FD], inv_sb[:, :FD])
                    pb = ps1[:, OFF2:OFF2 + FD]
                    nc.tensor.matmul(pb, lhsT=ones_rb[:], rhs=invb_sb[:, :FD],
                                     start=True, stop=True)
                    invbc = mpool.tile([128, OFF2], F32, tag="invbc")
                    nc.vector.tensor_copy(invbc[:, :FD], pb)
                    # normalize in SBUF (broadcast 1/s over the two c-halves
                    # via a step-0 middle dim), add residual, DMA out
                    nc.vector.tensor_mul(
                        out_sb[:, :, :FD], out_sb[:, :, :FD],
                        invbc[:, None, :FD].to_broadcast((128, 2, FD)))
                    nc.gpsimd.tensor_add(out_sb[:, :, :FD], out_sb[:, :, :FD],
                                         xf_sb[:, :, isl])
                    nc.sync.dma_start(out_r[:, :, isl], out_sb[:, :, :FD])

    # TRN2 allows at most one semaphore wait per instruction; Tile can emit
    # more. Split them (EventSemaphore chains) like Bacc.compile() does.
    _bass_rust.move_matmul_waits_to_ldweights(nc.m)
    _bass_rust.generate_event_semaphores(nc)
    return nc


_CACHED = {}


def _get_kernel(n_repeat: int = 1) -> bass.Bass:
    if n_repeat not in _CACHED:
        _CACHED[n_repeat] = build_kernel(n_repeat)
    return _CACHED[n_repeat]


def make_in_maps(x, Wq, bq, Wk, bk, Wv, bv, gamma):
    x = np.asarray(x, dtype=np.float32)
    Wq = np.asarray(Wq, dtype=np.float32)
    bq = np.asarray(bq, dtype=np.float32)
    Wk = np.asarray(Wk, dtype=np.float32)
    bk = np.asarray(bk, dtype=np.float32)
    Wv = np.asarray(Wv, dtype=np.float32)
    bv = np.asarray(bv, dtype=np.float32)
    g = float(np.asarray(gamma, dtype=np.float32).reshape(-1)[0])

    wqT = np.ascontiguousarray(Wq.T).astype(NP_BF16)            # [C, CI]
    wkT = np.ascontiguousarray(Wk.T).astype(NP_BF16)            # [C, CI]
    wvT = np.ascontiguousarray((g * Wv).T).astype(NP_BF16)      # [C, C]
    bq2 = np.ascontiguousarray(bq.reshape(CI, 1))               # [CI, 1] f32
    bk2 = np.ascontiguousarray(bk.reshape(CI, 1))

    # attention rows sum to 1 => V-bias contributes exactly gamma*bv per
    # pixel; fold it into the residual instead of a per-tile matmul
    xf = np.ascontiguousarray(
        x.reshape(B, C, N) + (g * bv)[None, :, None].astype(np.float32))
    xbf = np.ascontiguousarray(x.reshape(B, C, N)).astype(NP_BF16)

    in_maps = []
    for b in range(B):
        in_maps.append({
            "xb": xbf[b],
            "xf": xf[b],
            "wqT": wqT,
            "wkT": wkT,
            "wvT": wvT,
            "bq": bq2,
            "bk": bk2,
        })
    return in_maps


def kernel(x, Wq, bq, Wk, bk, Wv, bv, gamma):
    in_maps = make_in_maps(x, Wq, bq, Wk, bk, Wv, bv, gamma)
    nc = _get_kernel(1)
    res = run_bass_kernel_spmd(nc, in_maps, core_ids=list(range(NCORES)))
    out = np.stack([res.results[b]["out"] for b in range(B)], axis=0)
    return out.reshape(B, C, H, W).astype(np.float32)


# revision 16
# speedup vs baseline: 1.4008x; 1.4008x over previous
"""Trainium2 Bass kernel for PixelPropagationModule (per-pixel self-attention).

Math per batch sample b (B=8, C=256, CI=64, N=H*W=3136):
    Q = Wq @ x + bq            [CI, N]
    K = Wk @ x + bk            [CI, N]
    V = Wv @ x                 [C,  N]   (bias folded out, see below)
    score[i, j] = sum_o Q[o, i] K[o, j]          (N x N)
    att = softmax(score, axis=j)
    out = gamma * (V @ att^T) + (x + gamma*bv)   -> [C, N]

Since softmax rows sum to 1, the V bias contributes exactly gamma*bv to
every output pixel, so it is folded into the residual tensor on the host
(xf := x + gamma*bv) and V is computed bias-free on chip.

Sharding: pure data parallel, one sample per NeuronCore (B == 8 == n_cores).

Device dataflow (per core):
  - Everything is computed in the "transposed score" orientation S^T[j, i] so
    the attention weights come out of the PE array with j (the contraction
    index of the second matmul) on partitions; no on-chip transposes.
  - S^T matmuls contract over CI=64, which leaves half the 128-row PE array
    idle.  Fix: keep duplicated copies of Q and K on partitions 64..127
    (cheap SBUF->SBUF DMA) and row-tile two j-tiles concurrently - matmul A
    on PE rows 0..63 (tile_position (0,0)) computes j-tile 2s, matmul B on
    rows 64..127 (tile_position (64,0)) computes j-tile 2s+1, into different
    PSUM banks.  2x effective S throughput.
  - Q and K projections are col-tiled (Q -> psum partitions 0:64, K -> 64:128
    concurrently) since each has only M=64 output channels.
  - softmax without max subtraction (|score| <= ~40, exp safe in fp32/bf16);
    denominator s_i accumulated with DVE adds of the exp'ed tiles plus a
    final ones-vector matmul partition-reduce; normalization 1/s_i applied
    to the [C, N] output (deferred, flash-attention style).
  - exp on ACT reads PSUM pairs [128, 2, 448] (two j-tiles) per call to
    amortize the per-instruction overhead.

PSUM layout: S pair tiles are [128, 2, 448] with slots at element offsets
0 and 512 (bank-aligned, a matmul output must not cross a 2KiB psum bank);
out accumulators are [128, 1024] with the two C-halves at offsets 0 / 512.
"""

import numpy as np
import ml_dtypes

import bass_rust as _bass_rust

import concourse.bass as bass
import concourse.mybir as mybir
import concourse.tile as tile
from concourse.bass_utils import run_bass_kernel_spmd

BF16 = mybir.dt.bfloat16
F32 = mybir.dt.float32
NP_BF16 = ml_dtypes.bfloat16
AF = mybir.ActivationFunctionType

B, C, H, W = 8, 256, 56, 56
CI = 64
N = H * W            # 3136
NCORES = 8
FD = 448             # i-chunk width: 7 * 448 = 3136
OFF2 = 512           # second-slot offset inside [128, 1024] psum tiles
NJ = 25              # j-tiles: 24 x 128 + 1 x 64
NPAIR = 12           # full pairs of 128-wide j-tiles


def build_kernel(n_repeat: int = 1) -> bass.Bass:
    nc = bass.Bass()

    xb_d = nc.declare_dram_parameter("xb", [C, N], BF16, isOutput=False)
    xf_d = nc.declare_dram_parameter("xf", [C, N], F32, isOutput=False)
    wq_d = nc.declare_dram_parameter("wqT", [C, CI], BF16, isOutput=False)
    wk_d = nc.declare_dram_parameter("wkT", [C, CI], BF16, isOutput=False)
    wv_d = nc.declare_dram_parameter("wvT", [C, C], BF16, isOutput=False)
    bq_d = nc.declare_dram_parameter("bq", [CI, 1], F32, isOutput=False)
    bk_d = nc.declare_dram_parameter("bk", [CI, 1], F32, isOutput=False)
    out_d = nc.declare_dram_parameter("out", [C, N], F32, isOutput=True)

    xb_r = xb_d[:].rearrange("(o p) n -> p o n", p=128)    # [128, 2, N] bf16
    xf_r = xf_d[:].rearrange("(o p) n -> p o n", p=128)    # [128, 2, N] f32
    out_r = out_d[:].rearrange("(o p) n -> p o n", p=128)  # [128, 2, N] f32

    with tile.TileContext(nc) as tc:
        with (
            tc.tile_pool(name="const", bufs=1) as cpool,
            tc.tile_pool(name="data", bufs=1) as dpool,
            tc.tile_pool(name="att", bufs=6) as apool,
            tc.tile_pool(name="accp", bufs=2) as accpool,
            tc.tile_pool(name="outp", bufs=3) as opool,
            tc.tile_pool(name="misc", bufs=3) as mpool,
            tc.tile_pool(name="ps_a", bufs=2, space="PSUM") as ps_a,
            tc.tile_pool(name="ps_o", bufs=2, space="PSUM") as ps_o,
        ):
            # ---- constants / weights ----
            wq_sb = cpool.tile([128, 2, CI], BF16, name="wq_sb")
            nc.sync.dma_start(wq_sb[:], wq_d[:].rearrange("(o p) m -> p o m", p=128))
            wk_sb = cpool.tile([128, 2, CI], BF16, name="wk_sb")
            nc.sync.dma_start(wk_sb[:], wk_d[:].rearrange("(o p) m -> p o m", p=128))
            wv_sb = cpool.tile([128, 2, C], BF16, name="wv_sb")
            nc.sync.dma_start(wv_sb[:], wv_d[:].rearrange("(o p) m -> p o m", p=128))
            # Q bias on partitions 0:64, K bias on 64:128 — matching where
            # the col-tiled projection psum lives (ACT cannot shift lanes)
            bqk_sb = cpool.tile([128, 1], F32, name="bqk_sb")
            nc.sync.dma_start(bqk_sb[0:CI, :], bq_d[:])
            nc.sync.dma_start(bqk_sb[CI:128, :], bk_d[:])
            ones_col = cpool.tile([128, 1], BF16, name="ones_col")
            nc.vector.memset(ones_col[:], 1.0)
            ones_rb = cpool.tile([1, 128], BF16, name="ones_rb")
            nc.vector.memset(ones_rb[:], 1.0)


            # ---- x in SBUF (chunked so projections start early) ----
            xb_sb = dpool.tile([128, 2, N], BF16, name="xb_sb")
            xb_edges = [0, 112, 224, 448] + [448 * t for t in range(2, 8)]
            for e0, e1 in zip(xb_edges[:-1], xb_edges[1:]):
                nc.sync.dma_start(xb_sb[:, :, e0:e1], xb_r[:, :, e0:e1])
            xf_sb = dpool.tile([128, 2, N], F32, name="xf_sb")

            # warm the PE HAM clock gate during the initial x DMA wait:
            # dummy matmuls on a zeroed scratch tile (results never read)
            warm_sb = cpool.tile([128, 512], BF16, name="warm_sb")
            nc.vector.memset(warm_sb[:], 0.0)
            pwarm = ps_a.tile([128, 1024], F32, tag="ps_a")
            for wi in range(14):
                nc.tensor.matmul(pwarm[:, 0:512], lhsT=warm_sb[:, 0:128],
                                 rhs=warm_sb[:], start=True, stop=True)

            # residual input: needed only from the first chunk's tail on,
            # so emit after xb so it does not steal early DMA bandwidth
            nc.sync.dma_start(xf_sb[:], xf_r)

            # q2/k2: [128, N] with the projection on partitions 0:64 and a
            # DMA-duplicated copy on partitions 64:128 (for S row-tiling)
            q2_sb = dpool.tile([128, N], BF16, name="q2_sb")
            k2_sb = dpool.tile([128, N], BF16, name="k2_sb")
            vt_sb = dpool.tile([128, NJ, C], BF16, name="vt_sb")

            for _rep in range(n_repeat):
                # ---- projections, interleaved by x-DMA arrival ----
                # Q and K are col-tiled into one PE slot per x-half (Q ->
                # psum partitions 0:64, K -> 64:128); V^T tiles fully covered
                # by x columns [0, (t+1)*448) are emitted together so the PE
                # always has ready work while later x chunks stream in.
                vt_done = 0
                for t in range(N // FD):
                    sl = slice(t * FD, (t + 1) * FD)
                    pq = ps_a.tile([128, 1024], F32, tag="ps_a")
                    # Q at psum partitions 0:64 (cols 0:64), K at 64:128
                    nc.tensor.matmul(pq[0:CI, 0:FD], lhsT=wq_sb[:, 0, :],
                                     rhs=xb_sb[:, 0, sl], start=True, stop=False)
                    nc.tensor.matmul(pq[CI:128, 0:FD], lhsT=wk_sb[:, 0, :],
                                     rhs=xb_sb[:, 0, sl], start=True, stop=False)
                    nc.tensor.matmul(pq[0:CI, 0:FD], lhsT=wq_sb[:, 1, :],
                                     rhs=xb_sb[:, 1, sl], start=False, stop=True)
                    nc.tensor.matmul(pq[CI:128, 0:FD], lhsT=wk_sb[:, 1, :],
                                     rhs=xb_sb[:, 1, sl], start=False, stop=True)
                    # bias-apply on DVE (per-partition scalar AP), not ACT —
                    # ACT is the exp bottleneck, DVE has slack here
                    nc.vector.tensor_scalar_add(q2_sb[0:CI, sl],
                                                pq[0:CI, 0:FD], bqk_sb[0:CI, :])
                    nc.vector.tensor_scalar_add(k2_sb[CI:128, sl],
                                                pq[CI:128, 0:FD],
                                                bqk_sb[CI:128, :])
                    # duplicate across partition halves for S row-tiling
                    nc.sync.dma_start(q2_sb[CI:128, sl], q2_sb[0:CI, sl])
                    nc.sync.dma_start(k2_sb[0:CI, sl], k2_sb[CI:128, sl])
                    # V^T tiles: vt_sb[p, jt, c] = gamma*V[c, jt*128+p]
                    vt_avail = min(NJ, ((t + 1) * FD) // 128) if t < N // FD - 1 else NJ
                    for jt in range(vt_done, vt_avail):
                        jsz = 128 if jt < NJ - 1 else 64
                        j0 = jt * 128
                        pv = ps_a.tile([128, 1024], F32, tag="ps_a")
                        pvt = pv[:jsz, 0:C]
                        nc.tensor.matmul(pvt, lhsT=xb_sb[:, 0, j0:j0 + jsz],
                                         rhs=wv_sb[:, 0, :], start=True, stop=False)
                        nc.tensor.matmul(pvt, lhsT=xb_sb[:, 1, j0:j0 + jsz],
                                         rhs=wv_sb[:, 1, :], start=False, stop=True)
                        nc.vector.tensor_copy(vt_sb[:jsz, jt, :], pvt)
                    vt_done = vt_avail

                # ---- attention, one 448-wide query chunk at a time ----
                # S^T j-tiles are produced in row-tiled pairs: matmul A on PE
                # rows 0:64 -> psum slot 0, matmul B on rows 64:128 (the
                # duplicated q/k copies) -> psum slot 1 (different bank).
                # exp processes the pair in one ACT call.
                for ci in range(N // FD):
                    isl = slice(ci * FD, (ci + 1) * FD)
                    po = ps_o.tile([128, 1024], F32, tag="ps_o", name="po")
                    acc = accpool.tile([128, FD], BF16, tag="acc")

                    def emit_out_mms(jt2, att):
                        # att: [128, 2, FD] pair (j-tiles jt2, jt2+1) or
                        # [64, FD] single (last tile)
                        last = jt2 >= NJ - 1
                        if jt2 < NPAIR * 2:
                            for h in range(2):
                                jt = jt2 + h
                                for cc in range(2):
                                    vst = vt_sb[:, jt, cc * 128:(cc + 1) * 128]
                                    nc.tensor.matmul(
                                        po[:, cc * OFF2:cc * OFF2 + FD],
                                        lhsT=vst, rhs=att[:, h, :],
                                        start=(jt == 0), stop=(last and h == 1))
                        else:
                            for cc in range(2):
                                vst = vt_sb[:64, NJ - 1, cc * 128:(cc + 1) * 128]
                                nc.tensor.matmul(
                                    po[:, cc * OFF2:cc * OFF2 + FD],
                                    lhsT=vst, rhs=att[:64, 0, :],
                                    start=False, stop=True)

                    # software pipeline depth 2: out-matmuls of pair s are
                    # emitted after the S-matmuls of pair s+2, so the PE FIFO
                    # never waits on exp.
                    pending = []
                    for s in range(NPAIR + 1):
                        ps = ps_a.tile([128, 1024], F32, tag="ps_a")
                        if s < NPAIR:
                            jA, jB = 2 * s * 128, (2 * s + 1) * 128
                            # row-tiled concurrent pair (disjoint PE rows)
                            nc.tensor.matmul(ps[:, 0:FD],
                                             lhsT=k2_sb[0:64, jA:jA + 128],
                                             rhs=q2_sb[0:64, isl],
                                             start=True, stop=True)
                            nc.tensor.matmul(ps[:, OFF2:OFF2 + FD],
                                             lhsT=k2_sb[64:128, jB:jB + 128],
                                             rhs=q2_sb[64:128, isl],
                                             start=True, stop=True)
                            att = apool.tile([128, 2, FD], BF16, tag="att")
                        else:
                            # last j-tile (64 wide), single matmul
                            nc.tensor.matmul(ps[:64, 0:FD],
                                             lhsT=k2_sb[0:64, NPAIR * 256:N],
                                             rhs=q2_sb[0:64, isl],
                                             start=True, stop=True)
                            att = apool.tile([128, 2, FD], BF16, tag="att")
                        if len(pending) >= 2:
                            emit_out_mms(*pending.pop(0))
                        if s < NPAIR:
                            psv = ps[:].rearrange("p (h x) -> p h x", h=2)[:, :, 0:FD]
                            nc.scalar.activation(att[:], psv, AF.Exp)
                            # softmax denominator accumulation (DVE, 2x bf16)
                            if s == 0:
                                nc.vector.tensor_add(acc[:], att[:, 0, :],
                                                     att[:, 1, :])
                            else:
                                nc.vector.tensor_add(acc[:], acc[:], att[:, 0, :])
                                nc.vector.tensor_add(acc[:], acc[:], att[:, 1, :])
                            pending.append((2 * s, att))
                        else:
                            nc.scalar.activation(att[:64, 0, :], ps[:64, 0:FD],
                                                 AF.Exp)
                            att_last = att
                            pending.append((NJ - 1, att))
                    for p in pending:
                        emit_out_mms(*p)

                    # ---- softmax denominator + normalization + residual ----
                    out_sb = opool.tile([128, 2, OFF2], F32, tag="out")
                    # plain copies first: releases the po psum banks fast so
                    # the next chunk's out-matmuls can start
                    for cc in range(2):
                        nc.vector.tensor_copy(out_sb[:, cc, :FD],
                                              po[:, cc * OFF2:cc * OFF2 + FD])
                    ps1 = ps_o.tile([128, 1024], F32, tag="ps_o", name="ps1")
                    s1 = ps1[:1, 0:FD]
                    nc.tensor.matmul(s1, lhsT=ones_col[:], rhs=acc[:],
                                     start=True, stop=False)
                    nc.tensor.matmul(s1, lhsT=ones_col[:64],
                                     rhs=att_last[:64, 0, :],
                                     start=False, stop=True)
                    inv_sb = mpool.tile([1, OFF2], F32, tag="inv")
                    nc.vector.reciprocal(inv_sb[:, :FD], s1)
                    # bf16 copy of 1/s so the broadcast matmul runs at
                    # 1 cyc/row (fp32 operands cost 4 cyc/row on PE)
                    invb_sb = mpool.tile([1, OFF2], BF16, tag="invb")
                    nc.vector.tensor_copy(invb_sb[:, :FD], inv_sb[:, :FD])
                    pb = ps1[:, OFF2:OFF2 + FD]
                    nc.tensor.matmul(pb, lhsT=ones_rb[:], rhs=invb_sb[:, :FD],
                                     start=True, stop=True)
                    invbc = mpool.tile([128, OFF2], F32, tag="invbc")
                    nc.vector.tensor_copy(invbc[:, :FD], pb)
                    # normalize in SBUF (broadcast 1/s over the two c-halves
                    # via a step-0 middle dim), add residual, DMA out
                    nc.vector.tensor_mul(
                        out_sb[:, :, :FD], out_sb[:, :, :FD],
                        invbc[:, None, :FD].to_broadcast((128, 2, FD)))
                    nc.gpsimd.tensor_add(out_sb[:, :, :FD], out_sb[:, :, :FD],
                                         xf_sb[:, :, isl])
                    nc.sync.dma_start(out_r[:, :, isl], out_sb[:, :, :FD])

    # TRN2 allows at most one semaphore wait per instruction; Tile can emit
    # more. Split them (EventSemaphore chains) like Bacc.compile() does.
    _bass_rust.move_matmul_waits_to_ldweights(nc.m)
    _bass_rust.generate_event_semaphores(nc)
    return nc


_CACHED = {}


def _get_kernel(n_repeat: int = 1) -> bass.Bass:
    if n_repeat not in _CACHED:
        _CACHED[n_repeat] = build_kernel(n_repeat)
    return _CACHED[n_repeat]


def make_in_maps(x, Wq, bq, Wk, bk, Wv, bv, gamma):
    x = np.asarray(x, dtype=np.float32)
    Wq = np.asarray(Wq, dtype=np.float32)
    bq = np.asarray(bq, dtype=np.float32)
    Wk = np.asarray(Wk, dtype=np.float32)
    bk = np.asarray(bk, dtype=np.float32)
    Wv = np.asarray(Wv, dtype=np.float32)
    bv = np.asarray(bv, dtype=np.float32)
    g = float(np.asarray(gamma, dtype=np.float32).reshape(-1)[0])

    wqT = np.ascontiguousarray(Wq.T).astype(NP_BF16)            # [C, CI]
    wkT = np.ascontiguousarray(Wk.T).astype(NP_BF16)            # [C, CI]
    wvT = np.ascontiguousarray((g * Wv).T).astype(NP_BF16)      # [C, C]
    bq2 = np.ascontiguousarray(bq.reshape(CI, 1))               # [CI, 1] f32
    bk2 = np.ascontiguousarray(bk.reshape(CI, 1))

    # attention rows sum to 1 => V-bias contributes exactly gamma*bv per
    # pixel; fold it into the residual instead of a per-tile matmul
    xf = np.ascontiguousarray(
        x.reshape(B, C, N) + (g * bv)[None, :, None].astype(np.float32))
    xbf = np.ascontiguousarray(x.reshape(B, C, N)).astype(NP_BF16)

    in_maps = []
    for b in range(B):
        in_maps.append({
            "xb": xbf[b],
            "xf": xf[b],
            "wqT": wqT,
            "wkT": wkT,
            "wvT": wvT,
            "bq": bq2,
            "bk": bk2,
        })
    return in_maps


def kernel(x, Wq, bq, Wk, bk, Wv, bv, gamma):
    in_maps = make_in_maps(x, Wq, bq, Wk, bk, Wv, bv, gamma)
    nc = _get_kernel(1)
    res = run_bass_kernel_spmd(nc, in_maps, core_ids=list(range(NCORES)))
    out = np.stack([res.results[b]["out"] for b in range(B)], axis=0)
    return out.reshape(B, C, H, W).astype(np.float32)
